# revision 1
# baseline (speedup 1.0000x reference)
"""Trainium2 Bass kernel for nn_CachedCompressedLinear.

out[16, 11008] = x[16, 4096] @ ((w_q - 128) * scale).T + bias

Sharding: column-parallel over 8 NeuronCores. out_features padded
11008 -> 11264 = 8 * 1408; each core gets a [4096, 1408] int32 slice of
the (transposed) quantized weight, decodes it on-device (int32 -> bf16
with a -128 shift; integers <= 255 are exact in bf16), and computes its
[16, 1408] output slice.  x is replicated, pre-transposed and split into
bf16 hi/lo halves so the bf16 matmul carries fp32-level precision
(weights are exact in bf16, x_hi + x_lo represents x to ~2^-17).
The per-tensor scale and the bias are applied to the small f32 output
on-device in the epilogue.
"""

import sys

if "/opt/trn_rl_repo" not in sys.path:
    sys.path.insert(0, "/opt/trn_rl_repo")

import numpy as np
import ml_dtypes

IN_F = 4096
OUT_F = 11008
BATCH = 16
N_CORES = 8
O_PER = 1376  # out_features per core (11008 = 8 * 1376, no padding)
K_TILES = IN_F // 128  # 32
M = 48  # stationary columns: x_hi [0:16] | zeros [16:32] | x_lo [32:48]
# (PSUM partition reads must be 32-aligned, so lo lives at partition 32)
LO = 32
CHUNKS = [(0, 512), (512, 512), (1024, 352)]  # o-chunks within 1376

_BUILT = None


def _build():
    """Build the (SPMD, per-core) Bass program once."""
    import concourse.bass as bass
    import concourse.tile as tile
    from concourse import bacc, mybir

    dt = mybir.dt
    nc = bacc.Bacc("TRN2", target_bir_lowering=False, debug=False)

    wt = nc.dram_tensor("wt", [IN_F, O_PER], dt.int32, kind="ExternalInput")
    xt2 = nc.dram_tensor(
        "xt2", [128, (K_TILES + 1) * M], dt.bfloat16, kind="ExternalInput"
    )
    bias_rep = nc.dram_tensor(
        "bias_rep", [1, O_PER], dt.float32, kind="ExternalInput"
    )
    s_col = nc.dram_tensor("s_col", [BATCH, 1], dt.float32, kind="ExternalInput")
    out = nc.dram_tensor("out", [BATCH, O_PER], dt.float32, kind="ExternalOutput")

    PAIR = 2  # k-tiles per weight DMA (1.4 MB transfers)
    # group layout: pairs first, one single, then the final k-tile handled
    # chunk-wise (see below) so each output chunk closes as its slice lands
    GROUPS = [(g * PAIR, PAIR) for g in range(15)]  # k0..29; tail below
    BIASBLK = K_TILES  # extra stationary block holding the bias one-hot
    with tile.TileContext(nc) as tc:
        with (
            tc.tile_pool(name="consts", bufs=1) as consts,
            tc.tile_pool(name="w32", bufs=5) as w32p,
            tc.tile_pool(name="wbf", bufs=4) as wbfp,
            tc.tile_pool(name="psum", bufs=1, space=bass.MemorySpace.PSUM) as psump,
            tc.tile_pool(name="outp", bufs=1) as outp,
        ):
            # x (hi|lo) arrives host-prepacked in SBUF layout, plus one
            # extra block with the bias one-hot row: [128, 33*48]
            x_sb = consts.tile([128, (K_TILES + 1) * M], dt.bfloat16)
            nc.scalar.dma_start(x_sb[:], xt2[:])
            bias_sb = consts.tile([1, O_PER], dt.float32)
            nc.scalar.dma_start(bias_sb[:], bias_rep[:])
            s_sb = consts.tile([BATCH, 1], dt.float32)
            nc.scalar.dma_start(s_sb[:], s_col[:])

            # bias/s in bf16 hi/lo, fed to PSUM via two K=1 matmuls so the
            # epilogue does not need a separate bias add.
            rs = consts.tile([1, 1], dt.float32)
            nc.vector.reciprocal(rs[:], s_sb[0:1, 0:1])
            bq32 = consts.tile([1, O_PER], dt.float32)
            nc.vector.tensor_scalar_mul(bq32[:], bias_sb[0:1, :], rs[0:1, 0:1])
            bqh = consts.tile([1, O_PER], dt.bfloat16)
            nc.vector.tensor_copy(bqh[:], bq32[:])
            bql32 = consts.tile([1, O_PER], dt.float32)
            nc.vector.tensor_sub(bql32[:], bq32[:], bqh[:])
            bql = consts.tile([1, O_PER], dt.bfloat16)
            nc.vector.tensor_copy(bql[:], bql32[:])

            psums = [
                psump.tile([M, w], dt.float32, name=f"ps{i}", tag=f"ps{i}")
                for i, (_, w) in enumerate(CHUNKS)
            ]


            wt3 = wt[:].rearrange("(g p) f -> p g f", p=128)  # [128, 32, 1408]
            for k0, npk in GROUPS:
                wt_t = w32p.tile([128, PAIR, O_PER], dt.int32, tag="wt_t")
                nc.gpsimd.dma_start(
                    wt_t[:, 0:npk, :], wt3[:, k0 : k0 + npk, :]
                )
                wb_t = wbfp.tile([128, PAIR, O_PER], dt.bfloat16, tag="wb_t")
                for j in range(npk):
                    k = k0 + j
                    # decode: (codes - 128) cast to bf16 (exact for |v|<=256)
                    nc.vector.tensor_scalar_add(
                        wb_t[:, j, :], wt_t[:, j, :], -128.0
                    )
                    for i, (o, w) in enumerate(CHUNKS):
                        nc.tensor.matmul(
                            psums[i][:, :],
                            x_sb[:, k * M : (k + 1) * M],
                            wb_t[:, j, o : o + w],
                            start=(k == 0),
                            stop=False,
                        )
                    if k == 0:
                        # fold bias/s into the hi PSUM rows (K=1 matmuls)
                        for i, (o, w) in enumerate(CHUNKS):
                            for bvec in (bqh, bql):
                                nc.tensor.matmul(
                                    psums[i][:, :],
                                    x_sb[0:1, BIASBLK * M : (BIASBLK + 1) * M],
                                    bvec[0:1, o : o + w],
                                    start=False,
                                    stop=False,
                                )

            # final two k-tiles, chunk-wise and interleaved per chunk: each
            # output chunk's accumulation closes before the next chunk's data
            # arrives, so the PE queue at the last byte holds only the final
            # chunk's matmul (instead of ~5 queued cold matmuls)
            kA, kB = K_TILES - 2, K_TILES - 1
            wt_L = w32p.tile([128, PAIR, O_PER], dt.int32, tag="wt_t")
            wb_L = wbfp.tile([128, PAIR, O_PER], dt.bfloat16, tag="wb_t")
            for i, (o, w) in enumerate(CHUNKS):
                for j, kk in enumerate((kA, kB)):
                    nc.gpsimd.dma_start(
                        wt_L[:, j, o : o + w], wt3[:, kk, o : o + w]
                    )
                    nc.vector.tensor_scalar_add(
                        wb_L[:, j, o : o + w], wt_L[:, j, o : o + w], -128.0
                    )
                    nc.tensor.matmul(
                        psums[i][:, :],
                        x_sb[:, kk * M : (kk + 1) * M],
                        wb_L[:, j, o : o + w],
                        start=False,
                        stop=(kk == kB),
                    )

            for i, (o, w) in enumerate(CHUNKS):
                # hi -> ACT (Copy, scale fused); lo -> DVE (mul by s);
                # sum -> DVE; per-chunk output DMA. Bias is already in the
                # hi PSUM rows via the K=1 matmuls.
                his = outp.tile([BATCH, w], dt.float32, name=f"his{i}")
                nc.scalar.activation(
                    his[:],
                    psums[i][0:BATCH, :],
                    mybir.ActivationFunctionType.Copy,
                    scale=s_sb[:, 0:1],
                )
                los = outp.tile([BATCH, w], dt.float32, name=f"los{i}")
                nc.vector.tensor_scalar_mul(
                    los[:], psums[i][LO : LO + BATCH, :], s_sb[:, 0:1]
                )
                comb = outp.tile([BATCH, w], dt.float32, name=f"comb{i}")
                # all adds on DVE: GpSimd TT measured 2.2x slower and its
                # lateness blocked later out-DMAs through the sync FIFO
                nc.vector.tensor_add(comb[:], his[:], los[:])
                nc.sync.dma_start(out[:][:, o : o + w], comb[:])

    nc.compile()
    return nc


def _get_built():
    global _BUILT
    if _BUILT is None:
        _BUILT = _build()
    return _BUILT


def make_in_maps(x, w_q, scale, bias):
    """Host-side shard + layout prep. Returns per-core input dicts."""
    x = np.asarray(x, dtype=np.float32)
    w_q = np.asarray(w_q, dtype=np.int32)
    scale = np.asarray(scale, dtype=np.float32)
    bias = np.asarray(bias, dtype=np.float32)

    xT = np.ascontiguousarray(x.T)  # [4096, 16]
    xh = xT.astype(ml_dtypes.bfloat16)
    xl = (xT - xh.astype(np.float32)).astype(ml_dtypes.bfloat16)
    x48 = np.zeros((IN_F, M), dtype=ml_dtypes.bfloat16)  # [4096, 48]
    x48[:, :BATCH] = xh
    x48[:, LO : LO + BATCH] = xl
    # prepack to the SBUF layout [128, K_TILES*M]: partition p holds,
    # for each k-tile t, the stationary block row (t*128 + p)
    xt2 = np.zeros((128, (K_TILES + 1) * M), dtype=ml_dtypes.bfloat16)
    xt2[:, : K_TILES * M] = (
        x48.reshape(K_TILES, 128, M).transpose(1, 0, 2).reshape(128, K_TILES * M)
    )
    # bias one-hot block: partition 0, first BATCH stationary columns = 1
    xt2[0, K_TILES * M : K_TILES * M + BATCH] = 1.0

    s_col = np.full((BATCH, 1), scale.reshape(-1)[0], dtype=np.float32)

    in_maps = []
    for c in range(N_CORES):
        wt_c = np.ascontiguousarray(
            w_q[c * O_PER : (c + 1) * O_PER].T
        )  # [4096, 1376] int32
        bias_c = np.ascontiguousarray(
            bias[c * O_PER : (c + 1) * O_PER].reshape(1, O_PER)
        )
        in_maps.append(
            {"wt": wt_c, "xt2": xt2, "bias_rep": bias_c, "s_col": s_col}
        )
    return in_maps


def run(inputs, trace=False):
    """Run on the 8 NeuronCores. Returns (full_output, BassKernelResults)."""
    from concourse.bass_utils import run_bass_kernel_spmd

    in_maps = make_in_maps(**inputs)
    nc = _get_built()
    res = run_bass_kernel_spmd(nc, in_maps, list(range(N_CORES)), trace=trace)
    parts = [np.asarray(res.results[c]["out"]) for c in range(N_CORES)]
    full = np.concatenate(parts, axis=1)[:, :OUT_F].astype(np.float32)
    return full, res


def kernel(**inputs) -> np.ndarray:
    full, _ = run(inputs, trace=False)
    return full



# revision 3
# speedup vs baseline: 1.4717x; 1.4717x over previous
"""Trainium2 Bass kernel for nn_CachedCompressedLinear.

out[16, 11008] = x[16, 4096] @ ((w_q - 128) * scale).T + bias

Sharding: column-parallel over 8 NeuronCores; each core owns a 1376-wide
slice of out_features (8 * 1376 = 11008).

v2: the int32 weight codes are packed to uint8 on the host (values are
0..255, so the upper 3 bytes in HBM are zeros) cutting weight DMA 4x to
5.64 MB/core.  On-device decode uint8 -> bf16 (with the -128 shift fused)
is split between DVE (cols 0:864 of each k-tile, 2x perf mode) and ACT
(cols 864:1376, 1x) so it hides under the matmuls.  Matmuls run
back-to-back (progressive DMA group sizes fill the pipeline early) so the
PE stays at its warm 2.4 GHz clock; weights stream as bf16 moving data in
three PSUM chunks (512, 352, 512) per k-tile.  x is replicated,
pre-transposed and split into bf16 hi/lo halves so the bf16 matmul
carries fp32-level precision.  The per-tensor scale and the bias are
applied on the small f32 output in the epilogue (bias via K=1 matmuls
folded into the last k-tile's accumulation).
"""

import sys

if "/opt/trn_rl_repo" not in sys.path:
    sys.path.insert(0, "/opt/trn_rl_repo")

import numpy as np
import ml_dtypes

IN_F = 4096
OUT_F = 11008
BATCH = 16
N_CORES = 8
O_PER = 1376  # out_features per core
K_TILES = IN_F // 128  # 32
M = 48  # stationary columns: x_hi [0:16] | zeros [16:32] | x_lo [32:48]
LO = 32
# (offset, width, engine): DVE decodes [0, 864), ACT decodes [864, 1376)
CHUNKS = [(0, 512, "dve"), (512, 352, "dve"), (864, 512, "act")]
DVE_W = 864
ACT_W = 512
# k-tile group sizes: small first groups fill the pipeline quickly
GROUPS = [1, 1, 2, 4, 8, 8, 8]

_BUILT = None


def _build():
    """Build the (SPMD, per-core) Bass program once."""
    import concourse.bass as bass
    import concourse.tile as tile
    from concourse import bacc, mybir

    dt = mybir.dt
    nc = bacc.Bacc("TRN2", target_bir_lowering=False, debug=False)

    wt8 = nc.dram_tensor("wt8", [128, K_TILES * O_PER], dt.uint8,
                         kind="ExternalInput")
    xt2 = nc.dram_tensor(
        "xt2", [128, (K_TILES + 1) * M], dt.bfloat16, kind="ExternalInput"
    )
    bias_rep = nc.dram_tensor(
        "bias_rep", [1, O_PER], dt.float32, kind="ExternalInput"
    )
    s_col = nc.dram_tensor("s_col", [BATCH, 1], dt.float32, kind="ExternalInput")
    out = nc.dram_tensor("out", [BATCH, O_PER], dt.float32, kind="ExternalOutput")

    BIASBLK = K_TILES  # extra stationary block holding the bias one-hot
    with tile.TileContext(nc) as tc:
        with (
            tc.tile_pool(name="consts", bufs=1) as consts,
            tc.tile_pool(name="w8", bufs=3) as w8p,
            tc.tile_pool(name="wbA", bufs=3) as wbAp,
            tc.tile_pool(name="wbB", bufs=3) as wbBp,
            tc.tile_pool(name="psum", bufs=1, space=bass.MemorySpace.PSUM) as psump,
            tc.tile_pool(name="outp", bufs=1) as outp,
        ):
            # x (hi|lo) host-prepacked in SBUF layout + bias one-hot block
            x_sb = consts.tile([128, (K_TILES + 1) * M], dt.bfloat16)
            nc.gpsimd.dma_start(x_sb[:], xt2[:])
            bias_sb = consts.tile([1, O_PER], dt.float32)
            nc.gpsimd.dma_start(bias_sb[:], bias_rep[:])
            s_sb = consts.tile([BATCH, 1], dt.float32)
            nc.gpsimd.dma_start(s_sb[:], s_col[:])

            # bias/s in bf16 hi/lo, fed to PSUM via two K=1 matmuls in the
            # last k-tile so the epilogue needs no separate bias add.
            rs = consts.tile([1, 1], dt.float32)
            nc.vector.reciprocal(rs[:], s_sb[0:1, 0:1])
            bq32 = consts.tile([1, O_PER], dt.float32)
            nc.vector.tensor_scalar_mul(bq32[:], bias_sb[0:1, :], rs[0:1, 0:1])
            bqh = consts.tile([1, O_PER], dt.bfloat16)
            nc.vector.tensor_copy(bqh[:], bq32[:])
            bql32 = consts.tile([1, O_PER], dt.float32)
            nc.vector.tensor_sub(bql32[:], bq32[:], bqh[:])
            bql = consts.tile([1, O_PER], dt.bfloat16)
            nc.vector.tensor_copy(bql[:], bql32[:])

            psums = [
                psump.tile([M, w], dt.float32, name=f"ps{i}", tag=f"ps{i}")
                for i, (_, w, _e) in enumerate(CHUNKS)
            ]

            GMAX = max(GROUPS)
            k0 = 0
            for gi, G in enumerate(GROUPS):
                wt_t = w8p.tile([128, GMAX, O_PER], dt.uint8, tag="w8")
                nc.sync.dma_start(
                    wt_t[:, 0:G, :],
                    wt8[:, k0 * O_PER:(k0 + G) * O_PER],
                )
                # decode: DVE takes cols [0, 864), ACT takes [864, 1376)
                wbA = wbAp.tile([128, GMAX, DVE_W], dt.bfloat16, tag="wA")
                nc.vector.tensor_scalar_add(
                    wbA[:, 0:G, :], wt_t[:, 0:G, 0:DVE_W], -128.0
                )
                wbB = wbBp.tile([128, GMAX, ACT_W], dt.bfloat16, tag="wB")
                nc.scalar.activation(
                    wbB[:, 0:G, :], wt_t[:, 0:G, DVE_W:O_PER],
                    mybir.ActivationFunctionType.Copy, bias=-128.0
                )
                for t in range(G):
                    k = k0 + t
                    last = k == K_TILES - 1
                    if last:
                        # fold bias/s into the hi PSUM rows (K=1 matmuls)
                        for i, (o, w, _e) in enumerate(CHUNKS):
                            for bvec in (bqh, bql):
                                nc.tensor.matmul(
                                    psums[i][:, :],
                                    x_sb[0:1, BIASBLK * M:(BIASBLK + 1) * M],
                                    bvec[0:1, o:o + w],
                                    start=False,
                                    stop=False,
                                )
                    for i, (o, w, eng) in enumerate(CHUNKS):
                        if eng == "dve":
                            mv = wbA[:, t, o:o + w]
                        else:
                            mv = wbB[:, t, o - DVE_W:o - DVE_W + w]
                        nc.tensor.matmul(
                            psums[i][:, :],
                            x_sb[:, k * M:(k + 1) * M],
                            mv,
                            start=(k == 0),
                            stop=last,
                        )
                k0 += G

            for i, (o, w, _e) in enumerate(CHUNKS):
                # hi -> ACT (Copy, scale fused); lo -> DVE (mul by s);
                # sum -> DVE; per-chunk output DMA.
                his = outp.tile([BATCH, w], dt.float32, name=f"his{i}")
                nc.scalar.activation(
                    his[:],
                    psums[i][0:BATCH, :],
                    mybir.ActivationFunctionType.Copy,
                    scale=s_sb[:, 0:1],
                )
                los = outp.tile([BATCH, w], dt.float32, name=f"los{i}")
                nc.vector.tensor_scalar_mul(
                    los[:], psums[i][LO:LO + BATCH, :], s_sb[:, 0:1]
                )
                comb = outp.tile([BATCH, w], dt.float32, name=f"comb{i}")
                nc.vector.tensor_add(comb[:], his[:], los[:])
                nc.sync.dma_start(out[:][:, o:o + w], comb[:])

    nc.compile()
    return nc


def _get_built():
    global _BUILT
    if _BUILT is None:
        _BUILT = _build()
    return _BUILT


def make_in_maps(x, w_q, scale, bias):
    """Host-side shard + layout prep. Returns per-core input dicts."""
    x = np.asarray(x, dtype=np.float32)
    w_q = np.asarray(w_q, dtype=np.int32)
    scale = np.asarray(scale, dtype=np.float32)
    bias = np.asarray(bias, dtype=np.float32)

    xT = np.ascontiguousarray(x.T)  # [4096, 16]
    xh = xT.astype(ml_dtypes.bfloat16)
    xl = (xT - xh.astype(np.float32)).astype(ml_dtypes.bfloat16)
    x48 = np.zeros((IN_F, M), dtype=ml_dtypes.bfloat16)  # [4096, 48]
    x48[:, :BATCH] = xh
    x48[:, LO:LO + BATCH] = xl
    # prepack to the SBUF layout [128, K_TILES*M]: partition p holds,
    # for each k-tile t, the stationary block row (t*128 + p)
    xt2 = np.zeros((128, (K_TILES + 1) * M), dtype=ml_dtypes.bfloat16)
    xt2[:, :K_TILES * M] = (
        x48.reshape(K_TILES, 128, M).transpose(1, 0, 2).reshape(128, K_TILES * M)
    )
    # bias one-hot block: partition 0, first BATCH stationary columns = 1
    xt2[0, K_TILES * M:K_TILES * M + BATCH] = 1.0

    s_col = np.full((BATCH, 1), scale.reshape(-1)[0], dtype=np.float32)

    in_maps = []
    for c in range(N_CORES):
        # uint8 codes, transposed to [4096, 1376] then packed so partition
        # p holds, for k-tile t, row (t*128 + p): [128, 32*1376]
        wt_c = w_q[c * O_PER:(c + 1) * O_PER].T.astype(np.uint8)
        wt8_c = np.ascontiguousarray(
            wt_c.reshape(K_TILES, 128, O_PER)
            .transpose(1, 0, 2)
            .reshape(128, K_TILES * O_PER)
        )
        bias_c = np.ascontiguousarray(
            bias[c * O_PER:(c + 1) * O_PER].reshape(1, O_PER)
        )
        in_maps.append(
            {"wt8": wt8_c, "xt2": xt2, "bias_rep": bias_c, "s_col": s_col}
        )
    return in_maps


def run(inputs, trace=False):
    """Run on the 8 NeuronCores. Returns (full_output, BassKernelResults)."""
    from concourse.bass_utils import run_bass_kernel_spmd

    in_maps = make_in_maps(**inputs)
    nc = _get_built()
    res = run_bass_kernel_spmd(nc, in_maps, list(range(N_CORES)), trace=trace)
    parts = [np.asarray(res.results[c]["out"]) for c in range(N_CORES)]
    full = np.concatenate(parts, axis=1)[:, :OUT_F].astype(np.float32)
    return full, res


def kernel(**inputs) -> np.ndarray:
    full, _ = run(inputs, trace=False)
    return full


# revision 4
# speedup vs baseline: 1.5104x; 1.0263x over previous
"""Trainium2 Bass kernel for nn_CachedCompressedLinear.

out[16, 11008] = x[16, 4096] @ ((w_q - 128) * scale).T + bias

Sharding: column-parallel over 8 NeuronCores; each core owns a 1376-wide
slice of out_features (8 * 1376 = 11008).

v2: the int32 weight codes are packed to uint8 on the host (values are
0..255, so the upper 3 bytes in HBM are zeros) cutting weight DMA 4x to
5.64 MB/core.  On-device decode uint8 -> bf16 (with the -128 shift fused)
is split between DVE (cols 0:864 of each k-tile, 2x perf mode) and ACT
(cols 864:1376, 1x) so it hides under the matmuls.  Matmuls run
back-to-back (progressive DMA group sizes fill the pipeline early) so the
PE stays at its warm 2.4 GHz clock; weights stream as bf16 moving data in
three PSUM chunks (512, 352, 512) per k-tile.  x is replicated,
pre-transposed and split into bf16 hi/lo halves so the bf16 matmul
carries fp32-level precision.  The per-tensor scale and the bias are
applied on the small f32 output in the epilogue (bias via K=1 matmuls
folded into the last k-tile's accumulation).
"""

import sys

if "/opt/trn_rl_repo" not in sys.path:
    sys.path.insert(0, "/opt/trn_rl_repo")

import numpy as np
import ml_dtypes

IN_F = 4096
OUT_F = 11008
BATCH = 16
N_CORES = 8
O_PER = 1376  # out_features per core
K_TILES = IN_F // 128  # 32
M = 48  # stationary columns: x_hi [0:16] | zeros [16:32] | x_lo [32:48]
LO = 32
# (offset, width, engine): DVE decodes [0, 864), ACT decodes [864, 1376)
CHUNKS = [(0, 512, "dve"), (512, 352, "dve"), (864, 512, "act")]
DVE_W = 864
ACT_W = 512
# k-tile group sizes: small first groups fill the pipeline quickly
GROUPS = [1, 1, 2, 4, 8, 8, 8]

_BUILT = None


def _build():
    """Build the (SPMD, per-core) Bass program once."""
    import concourse.bass as bass
    import concourse.tile as tile
    from concourse import bacc, mybir

    dt = mybir.dt
    nc = bacc.Bacc("TRN2", target_bir_lowering=False, debug=False)

    wt8 = nc.dram_tensor("wt8", [128, K_TILES * O_PER], dt.uint8,
                         kind="ExternalInput")
    xt2 = nc.dram_tensor(
        "xt2", [128, (K_TILES + 1) * M], dt.bfloat16, kind="ExternalInput"
    )
    bias_rep = nc.dram_tensor(
        "bias_rep", [1, O_PER], dt.float32, kind="ExternalInput"
    )
    s_col = nc.dram_tensor("s_col", [BATCH, 1], dt.float32, kind="ExternalInput")
    out = nc.dram_tensor("out", [BATCH, O_PER], dt.float32, kind="ExternalOutput")

    BIASBLK = K_TILES  # extra stationary block holding the bias one-hot
    with tile.TileContext(nc) as tc:
        with (
            tc.tile_pool(name="consts", bufs=1) as consts,
            tc.tile_pool(name="w8", bufs=3) as w8p,
            tc.tile_pool(name="wbA", bufs=3) as wbAp,
            tc.tile_pool(name="wbB", bufs=3) as wbBp,
            tc.tile_pool(name="psum", bufs=1, space=bass.MemorySpace.PSUM) as psump,
            tc.tile_pool(name="outp", bufs=1) as outp,
        ):
            # x (hi|lo) host-prepacked in SBUF layout + bias one-hot block
            x_sb = consts.tile([128, (K_TILES + 1) * M], dt.bfloat16)
            nc.sync.dma_start(x_sb[:], xt2[:])
            bias_sb = consts.tile([1, O_PER], dt.float32)
            nc.sync.dma_start(bias_sb[:], bias_rep[:])
            s_sb = consts.tile([BATCH, 1], dt.float32)
            nc.sync.dma_start(s_sb[:], s_col[:])

            # bias/s in bf16 hi/lo, fed to PSUM via two K=1 matmuls in the
            # last k-tile so the epilogue needs no separate bias add.
            rs = consts.tile([1, 1], dt.float32)
            nc.vector.reciprocal(rs[:], s_sb[0:1, 0:1])
            bq32 = consts.tile([1, O_PER], dt.float32)
            nc.vector.tensor_scalar_mul(bq32[:], bias_sb[0:1, :], rs[0:1, 0:1])
            bqh = consts.tile([1, O_PER], dt.bfloat16)
            nc.vector.tensor_copy(bqh[:], bq32[:])
            bql32 = consts.tile([1, O_PER], dt.float32)
            nc.vector.tensor_sub(bql32[:], bq32[:], bqh[:])
            bql = consts.tile([1, O_PER], dt.bfloat16)
            nc.vector.tensor_copy(bql[:], bql32[:])

            psums = [
                psump.tile([M, w], dt.float32, name=f"ps{i}", tag=f"ps{i}")
                for i, (_, w, _e) in enumerate(CHUNKS)
            ]

            GMAX = max(GROUPS)
            k0 = 0
            for gi, G in enumerate(GROUPS):
                wt_t = w8p.tile([128, GMAX, O_PER], dt.uint8, tag="w8")
                nc.gpsimd.dma_start(
                    wt_t[:, 0:G, :],
                    wt8[:, k0 * O_PER:(k0 + G) * O_PER],
                )
                # decode: DVE takes cols [0, 864), ACT takes [864, 1376)
                wbA = wbAp.tile([128, GMAX, DVE_W], dt.bfloat16, tag="wA")
                nc.vector.tensor_scalar_add(
                    wbA[:, 0:G, :], wt_t[:, 0:G, 0:DVE_W], -128.0
                )
                wbB = wbBp.tile([128, GMAX, ACT_W], dt.bfloat16, tag="wB")
                nc.scalar.activation(
                    wbB[:, 0:G, :], wt_t[:, 0:G, DVE_W:O_PER],
                    mybir.ActivationFunctionType.Copy, bias=-128.0
                )
                for t in range(G):
                    k = k0 + t
                    last = k == K_TILES - 1
                    if last:
                        # fold bias/s into the hi PSUM rows (K=1 matmuls)
                        for i, (o, w, _e) in enumerate(CHUNKS):
                            for bvec in (bqh, bql):
                                nc.tensor.matmul(
                                    psums[i][:, :],
                                    x_sb[0:1, BIASBLK * M:(BIASBLK + 1) * M],
                                    bvec[0:1, o:o + w],
                                    start=False,
                                    stop=False,
                                )
                    for i, (o, w, eng) in enumerate(CHUNKS):
                        if eng == "dve":
                            mv = wbA[:, t, o:o + w]
                        else:
                            mv = wbB[:, t, o - DVE_W:o - DVE_W + w]
                        nc.tensor.matmul(
                            psums[i][:, :],
                            x_sb[:, k * M:(k + 1) * M],
                            mv,
                            start=(k == 0),
                            stop=last,
                        )
                k0 += G

            for i, (o, w, _e) in enumerate(CHUNKS):
                # hi -> ACT (Copy, scale fused); lo -> DVE (mul by s);
                # sum -> DVE; per-chunk output DMA.
                his = outp.tile([BATCH, w], dt.float32, name=f"his{i}")
                nc.scalar.activation(
                    his[:],
                    psums[i][0:BATCH, :],
                    mybir.ActivationFunctionType.Copy,
                    scale=s_sb[:, 0:1],
                )
                los = outp.tile([BATCH, w], dt.float32, name=f"los{i}")
                nc.vector.tensor_scalar_mul(
                    los[:], psums[i][LO:LO + BATCH, :], s_sb[:, 0:1]
                )
                comb = outp.tile([BATCH, w], dt.float32, name=f"comb{i}")
                nc.vector.tensor_add(comb[:], his[:], los[:])
                nc.sync.dma_start(out[:][:, o:o + w], comb[:])

    nc.compile()
    return nc


def _get_built():
    global _BUILT
    if _BUILT is None:
        _BUILT = _build()
    return _BUILT


def make_in_maps(x, w_q, scale, bias):
    """Host-side shard + layout prep. Returns per-core input dicts."""
    x = np.asarray(x, dtype=np.float32)
    w_q = np.asarray(w_q, dtype=np.int32)
    scale = np.asarray(scale, dtype=np.float32)
    bias = np.asarray(bias, dtype=np.float32)

    xT = np.ascontiguousarray(x.T)  # [4096, 16]
    xh = xT.astype(ml_dtypes.bfloat16)
    xl = (xT - xh.astype(np.float32)).astype(ml_dtypes.bfloat16)
    x48 = np.zeros((IN_F, M), dtype=ml_dtypes.bfloat16)  # [4096, 48]
    x48[:, :BATCH] = xh
    x48[:, LO:LO + BATCH] = xl
    # prepack to the SBUF layout [128, K_TILES*M]: partition p holds,
    # for each k-tile t, the stationary block row (t*128 + p)
    xt2 = np.zeros((128, (K_TILES + 1) * M), dtype=ml_dtypes.bfloat16)
    xt2[:, :K_TILES * M] = (
        x48.reshape(K_TILES, 128, M).transpose(1, 0, 2).reshape(128, K_TILES * M)
    )
    # bias one-hot block: partition 0, first BATCH stationary columns = 1
    xt2[0, K_TILES * M:K_TILES * M + BATCH] = 1.0

    s_col = np.full((BATCH, 1), scale.reshape(-1)[0], dtype=np.float32)

    in_maps = []
    for c in range(N_CORES):
        # uint8 codes, transposed to [4096, 1376] then packed so partition
        # p holds, for k-tile t, row (t*128 + p): [128, 32*1376]
        wt_c = w_q[c * O_PER:(c + 1) * O_PER].T.astype(np.uint8)
        wt8_c = np.ascontiguousarray(
            wt_c.reshape(K_TILES, 128, O_PER)
            .transpose(1, 0, 2)
            .reshape(128, K_TILES * O_PER)
        )
        bias_c = np.ascontiguousarray(
            bias[c * O_PER:(c + 1) * O_PER].reshape(1, O_PER)
        )
        in_maps.append(
            {"wt8": wt8_c, "xt2": xt2, "bias_rep": bias_c, "s_col": s_col}
        )
    return in_maps


def run(inputs, trace=False):
    """Run on the 8 NeuronCores. Returns (full_output, BassKernelResults)."""
    from concourse.bass_utils import run_bass_kernel_spmd

    in_maps = make_in_maps(**inputs)
    nc = _get_built()
    res = run_bass_kernel_spmd(nc, in_maps, list(range(N_CORES)), trace=trace)
    parts = [np.asarray(res.results[c]["out"]) for c in range(N_CORES)]
    full = np.concatenate(parts, axis=1)[:, :OUT_F].astype(np.float32)
    return full, res


def kernel(**inputs) -> np.ndarray:
    full, _ = run(inputs, trace=False)
    return full
